# revision 6
# baseline (speedup 1.0000x reference)
"""Trainium2 Bass kernel for nn_Add_Attn_Layer.

Computes out[b,i,j,c] = sum_d v[d] * tanh(start[b,c,i,d] + end[b,c,j,d])
for B=2, C=8, L=256, D=128 on 8 NeuronCores (2 (b,c) pairs per core).

Algorithm: separable Fourier expansion of tanh. With
tanh(z) ~= sum_k b_k sin(w_k z) and the addition theorem, each frequency
contributes two rank-128 accumulating PE matmuls per (i-half, pair):

  out[i,j] += sum_d [sin(w s_id)] * [b v_d cos(w e_jd)]
            + [cos(w s_id)] * [b v_d sin(w e_jd)]

FOUR frequencies {w1, 2*w1, w2, 2*w2} (weighted LSQ fit of tanh under
z~N(0,sqrt2) with the harmonic-pair constraint; end-to-end rel err
8.8e-3 vs the 2e-2 gate, bf16-emulated on the exact seeded inputs).
This is the balance point of the machine: each base frequency costs two
[128,1024] ACT Sin ops, each doubled frequency costs ~1.8us of DVE
products (double-angle identities), and every frequency costs 8
accumulating matmuls. 4 freqs halve the old 6-freq DVE load and cut
PE work 48->32 matmuls.

  sin(2w x) = 2 (sin cos),   cos(2w x) = 2 (cos^2 - 1/2)

where sp = sin*cos and cpp = cos^2 - 1/2 serve directly as stationaries
and the 2/4x factors and -1/2 affine fold into the per-partition fused
scale ops on the e-side.

ACT Sin has NO range reduction (accurate only |arg| <~ 3.93):
 - w1 = 0.465 is small enough that |w1 x| <= 2.36 and
   |w1 x + pi/2| <= 3.93 go to ACT directly (cos via +pi/2 bias).
 - w2 = 1.202 is range-reduced with ONE custom-DVE op (add_range_wrap):
   y = wrap(w2 x + pi/4) into [-pi, pi], then
   sin(w2 x) = Sin(y - pi/4), cos(w2 x) = Sin(y + pi/4), |args| <= 3.93.
   (The pi/4 pre-shift plays the same role as the old magic-number
   chain's 1/8 turn offset, at 1 DVE op instead of 3.)
The w2*x scaling runs on the otherwise-idle GPSIMD (Pool) engine, as
does one of the four bf16 double-angle products; two of the four PSUM
accumulator evacuations run on ACT (Copy) - all to keep DVE, the
bottleneck engine, minimal.
"""

from contextlib import ExitStack

import numpy as np

import concourse.bacc as bacc
import concourse.bass as bass
import concourse.tile as tile
from concourse import mybir
from concourse.bass_utils import run_bass_kernel_spmd
from concourse.masks import make_identity

B, C, L, D = 2, 8, 256, 128
N_CORES = 8
PAIRS = (B * C) // N_CORES  # (b,c) pairs per core = 2

F32 = mybir.dt.float32
BF16 = mybir.dt.bfloat16

PI = float(np.pi)
# tanh(z) ~= b1 sin(w1 z) + b2 sin(2 w1 z) + b3 sin(w2 z) + b4 sin(2 w2 z)
W1 = 0.465074
W2 = 1.201636
B1 = 1.28600417
B2 = -0.3218756
B3 = 0.43685613
B4 = 0.06056518

SE = PAIRS * L  # 512: columns of one tensor's (s or e) region
W = 2 * SE      # 1024: full basis-eval width (s of both pairs | e of both)


def build_nc(repeat=1, python_unroll=False):
    """repeat>1 re-emits the main loop (not the setup) in a For_i hardware
    loop for benchmarking: device time = setup + repeat * mainloop.
    python_unroll=True emits the repeats as straight-line code (for
    TimelineSim, which cannot resolve For_i branch registers)."""
    nc = bacc.Bacc("TRN2", target_bir_lowering=False, debug=False)

    s_ext = nc.declare_dram_parameter("start_hidden", [PAIRS, L, D], F32, isOutput=False)
    e_ext = nc.declare_dram_parameter("end_hidden", [PAIRS, L, D], F32, isOutput=False)
    v_ext = nc.declare_dram_parameter("v", [D, 1], F32, isOutput=False)
    # out[p, ih, il, j] = result(i=ih*128+il, j); host reshapes.
    out_ext = nc.declare_dram_parameter("out", [PAIRS, 2, 128, L], F32, isOutput=True)

    with ExitStack() as ctx:
        tc = ctx.enter_context(tile.TileContext(nc))
        singles = ctx.enter_context(tc.tile_pool(name="singles", bufs=1))
        setup = ctx.enter_context(tc.tile_pool(name="setup", bufs=2))
        tpool = ctx.enter_context(tc.tile_pool(name="tpool", bufs=2))
        psum = ctx.enter_context(tc.tile_pool(name="psum", bufs=2, space="PSUM"))
        accp = ctx.enter_context(tc.tile_pool(name="accp", bufs=1, space="PSUM"))

        # ---- setup: transpose s, e to [d, cols] via PE into one tile ----
        # se_all cols: [s_p0 | s_p1 | e_p0 | e_p1], 256 each.
        ident = singles.tile([128, 128], F32)
        make_identity(nc, ident)
        se_all = singles.tile([D, W], F32)
        nat_s = setup.tile([128, PAIRS, 2, D], F32, tag="nat_s")
        nat_e = setup.tile([128, PAIRS, 2, D], F32, tag="nat_e")
        # one DMA per tensor (each dma_start costs ~565ns of serialized
        # issue time at startup)
        for src, dst_t in ((s_ext, nat_s), (e_ext, nat_e)):
            nc.sync.dma_start(
                out=dst_t,
                in_=src.rearrange("p (h i) d -> i p h d", i=128))

        v32 = singles.tile([D, 1], F32)
        nc.sync.dma_start(out=v32, in_=v_ext[:, :])
        # per-partition scale vectors for the e-side fused ops
        bv1 = singles.tile([D, 1], F32)   # B1 v      (base w1)
        bv3 = singles.tile([D, 1], F32)   # B3 v      (base w2)
        a2 = singles.tile([D, 1], F32)    # 4 B2 v    (double 2w1: mult)
        n2 = singles.tile([D, 1], F32)    # -2 B2 v   (double 2w1: add)
        a4 = singles.tile([D, 1], F32)    # 4 B4 v    (double 2w2: mult)
        n4 = singles.tile([D, 1], F32)    # -2 B4 v   (double 2w2: add)
        for dst, scl in ((bv1, B1), (bv3, B3), (a2, 4.0 * B2), (n2, -2.0 * B2),
                         (a4, 4.0 * B4), (n4, -2.0 * B4)):
            nc.vector.tensor_scalar_mul(out=dst, in0=v32, scalar1=float(scl))
        halfpi = singles.tile([128, 1], F32)
        nc.gpsimd.memset(halfpi, PI / 2)
        bias_sin = singles.tile([128, 1], F32)
        nc.gpsimd.memset(bias_sin, -PI / 4)
        bias_cos = singles.tile([128, 1], F32)
        nc.gpsimd.memset(bias_cos, PI / 4)

        for half, nat in ((0, nat_s), (1, nat_e)):
            for p in range(PAIRS):
                for h in range(2):
                    tr = psum.tile([128, 128], F32, tag="tr")
                    nc.tensor.transpose(tr, nat[:, p, h, :], ident)
                    c0 = half * SE + p * L + h * 128
                    nc.vector.tensor_copy(out=se_all[:, c0:c0 + 128], in_=tr)

        # ---- main loop ----
        def main_body():
            accs = [
                [
                    accp.tile([128, L], F32, tag=f"acc{p}{ih}",
                              name=f"acc{p}{ih}")
                    for ih in range(2)
                ]
                for p in range(PAIRS)
            ]

            # --- range reduction for w2 (Pool feeds DVE feeds ACT) ---
            t2 = tpool.tile([D, W], F32, tag="t2", name="t2")
            nc.gpsimd.tensor_scalar_mul(out=t2, in0=se_all, scalar1=W2)
            w2a = tpool.tile([D, W], F32, tag="w2a", name="w2a")
            nc.vector.add_range_wrap(
                out=w2a, in_=t2, shift=PI / 4, bound=PI, period=2 * PI)

            # --- 4 ACT Sin ops: base sin/cos for w1 (direct) and w2 ---
            sin1 = tpool.tile([D, W], BF16, tag="sin1", name="sin1")
            nc.scalar.activation(
                out=sin1, in_=se_all,
                func=mybir.ActivationFunctionType.Sin, scale=W1)
            cos1 = tpool.tile([D, W], BF16, tag="cos1", name="cos1")
            nc.scalar.activation(
                out=cos1, in_=se_all,
                func=mybir.ActivationFunctionType.Sin, scale=W1, bias=halfpi)
            sin2 = tpool.tile([D, W], BF16, tag="sin2", name="sin2")
            nc.scalar.activation(
                out=sin2, in_=w2a,
                func=mybir.ActivationFunctionType.Sin, bias=bias_sin)
            cos2 = tpool.tile([D, W], BF16, tag="cos2", name="cos2")
            nc.scalar.activation(
                out=cos2, in_=w2a,
                func=mybir.ActivationFunctionType.Sin, bias=bias_cos)

            # --- double-angle products (bf16) ---
            sp1 = tpool.tile([D, W], BF16, tag="sp1", name="sp1")
            nc.vector.tensor_tensor(
                out=sp1, in0=sin1, in1=cos1, op=mybir.AluOpType.mult)
            sq1 = tpool.tile([D, W], BF16, tag="sq1", name="sq1")
            nc.gpsimd.tensor_tensor(
                out=sq1, in0=cos1, in1=cos1, op=mybir.AluOpType.mult)
            sp2 = tpool.tile([D, W], BF16, tag="sp2", name="sp2")
            nc.vector.tensor_tensor(
                out=sp2, in0=sin2, in1=cos2, op=mybir.AluOpType.mult)
            sq2 = tpool.tile([D, W], BF16, tag="sq2", name="sq2")
            nc.vector.tensor_tensor(
                out=sq2, in0=cos2, in1=cos2, op=mybir.AluOpType.mult)

            # --- stationary-side trims (s half) ---
            cpp1 = tpool.tile([D, SE], BF16, tag="cpp1", name="cpp1")
            nc.vector.tensor_scalar(
                out=cpp1, in0=sq1[:, 0:SE], scalar1=-0.5, scalar2=None,
                op0=mybir.AluOpType.add)
            cpp2 = tpool.tile([D, SE], BF16, tag="cpp2", name="cpp2")
            nc.vector.tensor_scalar(
                out=cpp2, in0=sq2[:, 0:SE], scalar1=-0.5, scalar2=None,
                op0=mybir.AluOpType.add)

            # --- e-side scaled moving tiles ---
            ecos1 = tpool.tile([D, SE], BF16, tag="ecos1", name="ecos1")
            nc.vector.tensor_scalar_mul(
                out=ecos1, in0=cos1[:, SE:W], scalar1=bv1)
            esin1 = tpool.tile([D, SE], BF16, tag="esin1", name="esin1")
            nc.vector.tensor_scalar_mul(
                out=esin1, in0=sin1[:, SE:W], scalar1=bv1)
            ecos2 = tpool.tile([D, SE], BF16, tag="ecos2", name="ecos2")
            nc.vector.tensor_scalar_mul(
                out=ecos2, in0=cos2[:, SE:W], scalar1=bv3)
            esin2 = tpool.tile([D, SE], BF16, tag="esin2", name="esin2")
            nc.vector.tensor_scalar_mul(
                out=esin2, in0=sin2[:, SE:W], scalar1=bv3)
            r11 = tpool.tile([D, SE], BF16, tag="r11", name="r11")
            nc.vector.tensor_scalar(
                out=r11, in0=sq1[:, SE:W], scalar1=a2, scalar2=n2,
                op0=mybir.AluOpType.mult, op1=mybir.AluOpType.add)
            r21 = tpool.tile([D, SE], BF16, tag="r21", name="r21")
            nc.vector.tensor_scalar_mul(out=r21, in0=sp1[:, SE:W], scalar1=a2)
            r12 = tpool.tile([D, SE], BF16, tag="r12", name="r12")
            nc.vector.tensor_scalar(
                out=r12, in0=sq2[:, SE:W], scalar1=a4, scalar2=n4,
                op0=mybir.AluOpType.mult, op1=mybir.AluOpType.add)
            r22 = tpool.tile([D, SE], BF16, tag="r22", name="r22")
            nc.vector.tensor_scalar_mul(out=r22, in0=sp2[:, SE:W], scalar1=a4)

            # --- 32 accumulating matmuls: 4 term-pairs x (pair, i-half) ---
            # term order: (w1 base), (2w1 double), (w2 base), (2w2 double)
            terms = [
                (sin1, ecos1, cos1, esin1),
                (sp1, r11, cpp1, r21),
                (sin2, ecos2, cos2, esin2),
                (sp2, r12, cpp2, r22),
            ]
            ntt = len(terms)
            for t_i, (lhs_a, rhs_a, lhs_b, rhs_b) in enumerate(terms):
                for p in range(PAIRS):
                    for ih in range(2):
                        # s-region starts at col 0 in both [D,W] and [D,SE]
                        # tiles, so the same slice works for either width.
                        sl = slice(p * L + ih * 128, p * L + (ih + 1) * 128)
                        # e-side tiles are [D, SE]; take this pair's L cols
                        ecs = slice(p * L, (p + 1) * L)
                        nc.tensor.matmul(
                            accs[p][ih], lhsT=lhs_a[:, sl],
                            rhs=rhs_a[:, ecs], start=(t_i == 0), stop=False)
                        nc.tensor.matmul(
                            accs[p][ih], lhsT=lhs_b[:, sl],
                            rhs=rhs_b[:, ecs],
                            start=False, stop=(t_i == ntt - 1))

            # --- evacuate PSUM (2 on DVE, 2 on ACT) + output DMA ---
            for p in range(PAIRS):
                for ih in range(2):
                    ev = setup.tile([128, L], F32, tag=f"ev{p}{ih}", name="ev")
                    if ih == 0:
                        nc.vector.tensor_copy(out=ev, in_=accs[p][ih])
                    else:
                        nc.scalar.copy(out=ev, in_=accs[p][ih])
                    nc.sync.dma_start(out=out_ext[p, ih], in_=ev)

        if repeat == 1:
            main_body()
        elif python_unroll:
            for _ in range(repeat):
                main_body()
        else:
            with tc.For_i(0, repeat, 1):
                main_body()
    nc.compile()
    return nc


_NC_CACHE = None


def kernel(start_hidden, end_hidden, v):
    global _NC_CACHE
    if _NC_CACHE is None:
        _NC_CACHE = build_nc()
    nc = _NC_CACHE

    sh = np.ascontiguousarray(start_hidden, dtype=np.float32).reshape(B * C, L, D)
    eh = np.ascontiguousarray(end_hidden, dtype=np.float32).reshape(B * C, L, D)
    v2 = np.ascontiguousarray(v, dtype=np.float32).reshape(D, 1)

    in_maps = [
        {
            "start_hidden": sh[k * PAIRS:(k + 1) * PAIRS],
            "end_hidden": eh[k * PAIRS:(k + 1) * PAIRS],
            "v": v2,
        }
        for k in range(N_CORES)
    ]

    res = None
    for attempt in range(3):
        try:
            res = run_bass_kernel_spmd(nc, in_maps, core_ids=list(range(N_CORES)))
            break
        except Exception:
            # transient NRT device-unrecoverable states clear on retry
            if attempt == 2:
                raise
            import time as _t
            _t.sleep(5)
    # per-core out: [PAIRS, 2, 128, L] = [p, ih, il, j] -> [p, i, j]
    per_core = [
        res.results[k]["out"].reshape(PAIRS, L, L)
        for k in range(N_CORES)
    ]
    full = np.concatenate(per_core, axis=0)  # [B*C, L(i), L(j)] in (b,c) order
    return np.ascontiguousarray(
        full.reshape(B, C, L, L).transpose(0, 2, 3, 1)
    ).astype(np.float32)


# revision 14
# speedup vs baseline: 1.8715x; 1.8715x over previous
"""Trainium2 Bass kernel for nn_Add_Attn_Layer.

Computes out[b,i,j,c] = sum_d v[d] * tanh(start[b,c,i,d] + end[b,c,j,d])
for B=2, C=8, L=256, D=128 on 8 NeuronCores (2 (b,c) pairs per core).

Algorithm: separable Fourier expansion of tanh. With
tanh(z) ~= sum_k b_k sin(w_k z) and the addition theorem, each frequency
contributes two rank-128 accumulating PE matmuls per (i-half, pair):

  out[i,j] += sum_d [sin(w s_id)] * [b v_d cos(w e_jd)]
            + [cos(w s_id)] * [b v_d sin(w e_jd)]

FOUR frequencies {w1, 2*w1, w2, 2*w2} (weighted LSQ fit of tanh under
z~N(0,sqrt2) with the harmonic-pair constraint; end-to-end rel err
8.8e-3 vs the 2e-2 gate, validated bf16-emulated on the exact seeded
inputs and on silicon). This is the balance point of the machine: each
base frequency costs two [128,1024] ACT Sin ops (~1.15us each on HW),
each doubled frequency ~1.3us of DVE double-angle products, and every
frequency 8 accumulating matmuls. 4 freqs halve the old 6-freq DVE load
and cut PE work 48->32 matmuls.

  sin(2w x) = 2 (sin cos),   cos(2w x) = 2 (cos^2 - 1/2)

sp = sin*cos and cpp = cos^2 - 1/2 serve directly as stationaries; the
2/4x factors and the -1/2 affine fold into the per-partition fused
scale ops on the e-side.

ACT Sin has NO range reduction (accurate only |arg| <~ 3.93):
 - w1 = 0.465 is small enough that |w1 x| <= 2.36 and
   |w1 x + pi/2| <= 3.93 go to ACT directly (cos via +pi/2 bias).
 - w2 = 1.202 is range-reduced with ONE custom-DVE op (add_range_wrap):
   y = wrap(w2 x + pi/4) into [-pi, pi], then
   sin(w2 x) = Sin(y - pi/4), cos(w2 x) = Sin(y + pi/4), |args| <= 3.93.
   (The pi/4 pre-shift plays the same role as the old magic-number
   chain's 1/8 turn offset, at 1 DVE op instead of 3.)

Layout tricks for fewer/wider DVE ops (HW shows ~110ns fixed cost per
op plus a semaphore hop on every dependence edge):
 - each freq's sin and cos land in halves of ONE [D, 2W] tile, so the
   e-side b*v scaling of both is a single strided TensorScalarPtr;
   likewise sp|sq land in one tile.
 - the two i-half accumulators of a pair share one PSUM bank
   ([128, 512] tile), so evacuation is 2 wide copies + 2 DMAs instead
   of 4+4, and the accumulator pool double-buffers across For_i
   iterations.

GPSIMD (Pool) tensor ops measure ~2us per [128,1024] op on HW and
serialize badly in-loop - everything elementwise stays on DVE/ACT.
"""

from contextlib import ExitStack

import numpy as np

import concourse.bacc as bacc
import concourse.bass as bass
import concourse.tile as tile
from concourse import mybir
from concourse.bass_utils import run_bass_kernel_spmd
from concourse.masks import make_identity

B, C, L, D = 2, 8, 256, 128
N_CORES = 8
PAIRS = (B * C) // N_CORES  # (b,c) pairs per core = 2

F32 = mybir.dt.float32
BF16 = mybir.dt.bfloat16

PI = float(np.pi)
# tanh(z) ~= b1 sin(w1 z) + b2 sin(2 w1 z) + b3 sin(w2 z) + b4 sin(2 w2 z)
W1 = 0.465074
W2 = 1.201636
B1 = 1.28600417
B2 = -0.3218756
B3 = 0.43685613
B4 = 0.06056518

SE = PAIRS * L  # 512: columns of one tensor's (s or e) region
W = 2 * SE      # 1024: full basis-eval width (s of both pairs | e of both)


def build_nc(repeat=1, python_unroll=False):
    """repeat>1 re-emits the main loop (not the setup) in a For_i hardware
    loop for benchmarking: device time = setup + repeat * mainloop.
    python_unroll=True emits the repeats as straight-line code (for
    TimelineSim, which cannot resolve For_i branch registers)."""
    nc = bacc.Bacc("TRN2", target_bir_lowering=False, debug=False)

    s_ext = nc.declare_dram_parameter("start_hidden", [PAIRS, L, D], F32, isOutput=False)
    e_ext = nc.declare_dram_parameter("end_hidden", [PAIRS, L, D], F32, isOutput=False)
    v_ext = nc.declare_dram_parameter("v", [D, 1], F32, isOutput=False)
    # out[p, ih, il, j] = result(i=ih*128+il, j); host reshapes.
    out_ext = nc.declare_dram_parameter("out", [PAIRS, 2, 128, L], F32, isOutput=True)

    with ExitStack() as ctx:
        tc = ctx.enter_context(tile.TileContext(nc))
        singles = ctx.enter_context(tc.tile_pool(name="singles", bufs=1))
        setup = ctx.enter_context(tc.tile_pool(name="setup", bufs=2))
        tpool = ctx.enter_context(tc.tile_pool(name="tpool", bufs=3))
        psum = ctx.enter_context(tc.tile_pool(name="psum", bufs=2, space="PSUM"))
        accp = ctx.enter_context(tc.tile_pool(name="accp", bufs=2, space="PSUM"))

        # ---- setup: transpose s, e to [d, cols] via PE into one tile ----
        # se_all cols: [s_p0 | s_p1 | e_p0 | e_p1], 256 each.
        ident = singles.tile([128, 128], F32)
        make_identity(nc, ident)
        se_all = singles.tile([D, W], F32)
        nat_s = setup.tile([128, PAIRS, 2, D], F32, tag="nat_s")
        nat_e = setup.tile([128, PAIRS, 2, D], F32, tag="nat_e")
        # one DMA per tensor (each dma_start costs ~565ns of serialized
        # issue time at startup)
        for src, dst_t in ((s_ext, nat_s), (e_ext, nat_e)):
            nc.sync.dma_start(
                out=dst_t,
                in_=src.rearrange("p (h i) d -> i p h d", i=128))

        v32 = singles.tile([D, 1], F32)
        nc.sync.dma_start(out=v32, in_=v_ext[:, :])
        # per-partition scale vectors for the e-side fused ops
        bv1 = singles.tile([D, 1], F32)   # B1 v      (base w1)
        bv3 = singles.tile([D, 1], F32)   # B3 v      (base w2)
        a2 = singles.tile([D, 1], F32)    # 4 B2 v    (double 2w1: mult)
        n2 = singles.tile([D, 1], F32)    # -2 B2 v   (double 2w1: add)
        a4 = singles.tile([D, 1], F32)    # 4 B4 v    (double 2w2: mult)
        n4 = singles.tile([D, 1], F32)    # -2 B4 v   (double 2w2: add)
        for dst, scl in ((bv1, B1), (bv3, B3), (a2, 4.0 * B2), (n2, -2.0 * B2),
                         (a4, 4.0 * B4), (n4, -2.0 * B4)):
            nc.vector.tensor_scalar_mul(out=dst, in0=v32, scalar1=float(scl))
        halfpi = singles.tile([128, 1], F32)
        nc.gpsimd.memset(halfpi, PI / 2)
        bias_sin = singles.tile([128, 1], F32)
        nc.gpsimd.memset(bias_sin, -PI / 4)
        bias_cos = singles.tile([128, 1], F32)
        nc.gpsimd.memset(bias_cos, PI / 4)

        for half, nat in ((0, nat_s), (1, nat_e)):
            for p in range(PAIRS):
                for h in range(2):
                    tr = psum.tile([128, 128], F32, tag="tr")
                    nc.tensor.transpose(tr, nat[:, p, h, :], ident)
                    c0 = half * SE + p * L + h * 128
                    nc.vector.tensor_copy(out=se_all[:, c0:c0 + 128], in_=tr)

        # ---- main loop ----
        def main_body():
            # acc2[p]: both i-halves of pair p in ONE [128, 512] PSUM tile
            # (cols ih*256+j) = one full 2KB bank; evac = 1 wide copy+DMA.
            acc2 = [accp.tile([128, 2 * L], F32, tag=f"acc{p}",
                              name=f"acc{p}") for p in range(PAIRS)]

            # --- range reduction for w2: y = wrap(w2 x + pi/4) ---
            t2 = tpool.tile([D, W], F32, tag="t2", name="t2")
            nc.vector.tensor_scalar_mul(out=t2, in0=se_all, scalar1=W2)
            w2a = tpool.tile([D, W], F32, tag="w2a", name="w2a")
            nc.vector.add_range_wrap(
                out=w2a, in_=t2, shift=PI / 4, bound=PI, period=2 * PI)

            # --- 4 ACT Sin ops; each freq's sin|cos in halves of one tile ---
            sc1 = tpool.tile([D, 2, W], BF16, tag="sc1", name="sc1")
            nc.scalar.activation(
                out=sc1[:, 0, :], in_=se_all,
                func=mybir.ActivationFunctionType.Sin, scale=W1)
            nc.scalar.activation(
                out=sc1[:, 1, :], in_=se_all,
                func=mybir.ActivationFunctionType.Sin, scale=W1, bias=halfpi)
            sc2 = tpool.tile([D, 2, W], BF16, tag="sc2", name="sc2")
            nc.scalar.activation(
                out=sc2[:, 0, :], in_=w2a,
                func=mybir.ActivationFunctionType.Sin, bias=bias_sin)
            nc.scalar.activation(
                out=sc2[:, 1, :], in_=w2a,
                func=mybir.ActivationFunctionType.Sin, bias=bias_cos)
            sin1, cos1 = sc1[:, 0, :], sc1[:, 1, :]
            sin2, cos2 = sc2[:, 0, :], sc2[:, 1, :]

            # --- double-angle products: sp|sq of each freq in one tile ---
            pq1 = tpool.tile([D, 2, W], BF16, tag="pq1", name="pq1")
            nc.vector.tensor_tensor(
                out=pq1[:, 0, :], in0=sin1, in1=cos1, op=mybir.AluOpType.mult)
            nc.vector.tensor_tensor(
                out=pq1[:, 1, :], in0=cos1, in1=cos1, op=mybir.AluOpType.mult)
            pq2 = tpool.tile([D, 2, W], BF16, tag="pq2", name="pq2")
            nc.vector.tensor_tensor(
                out=pq2[:, 0, :], in0=sin2, in1=cos2, op=mybir.AluOpType.mult)
            nc.vector.tensor_tensor(
                out=pq2[:, 1, :], in0=cos2, in1=cos2, op=mybir.AluOpType.mult)
            sp1, sq1 = pq1[:, 0, :], pq1[:, 1, :]
            sp2, sq2 = pq2[:, 0, :], pq2[:, 1, :]

            # --- stationary-side trims (s half) ---
            cpp1 = tpool.tile([D, SE], BF16, tag="cpp1", name="cpp1")
            nc.vector.tensor_scalar(
                out=cpp1, in0=sq1[:, 0:SE], scalar1=-0.5, scalar2=None,
                op0=mybir.AluOpType.add)
            cpp2 = tpool.tile([D, SE], BF16, tag="cpp2", name="cpp2")
            nc.vector.tensor_scalar(
                out=cpp2, in0=sq2[:, 0:SE], scalar1=-0.5, scalar2=None,
                op0=mybir.AluOpType.add)

            # --- e-side scaled moving tiles ---
            # base freqs: ONE strided TSPtr covers sin and cos e-halves
            esc1 = tpool.tile([D, 2, SE], BF16, tag="esc1", name="esc1")
            nc.vector.tensor_scalar_mul(
                out=esc1[:, 0, :], in0=sin1[:, SE:W], scalar1=bv1)
            nc.vector.tensor_scalar_mul(
                out=esc1[:, 1, :], in0=cos1[:, SE:W], scalar1=bv1)
            esc2 = tpool.tile([D, 2, SE], BF16, tag="esc2", name="esc2")
            nc.vector.tensor_scalar_mul(
                out=esc2[:, 0, :], in0=sin2[:, SE:W], scalar1=bv3)
            nc.vector.tensor_scalar_mul(
                out=esc2[:, 1, :], in0=cos2[:, SE:W], scalar1=bv3)
            esin1, ecos1 = esc1[:, 0, :], esc1[:, 1, :]
            esin2, ecos2 = esc2[:, 0, :], esc2[:, 1, :]
            # doubles: r2=sp_e*a (sin-like), r1=sq_e*a + n (cos-like)
            r11 = tpool.tile([D, SE], BF16, tag="r11", name="r11")
            nc.vector.tensor_scalar(
                out=r11, in0=sq1[:, SE:W], scalar1=a2, scalar2=n2,
                op0=mybir.AluOpType.mult, op1=mybir.AluOpType.add)
            r21 = tpool.tile([D, SE], BF16, tag="r21", name="r21")
            nc.vector.tensor_scalar_mul(out=r21, in0=sp1[:, SE:W], scalar1=a2)
            r12 = tpool.tile([D, SE], BF16, tag="r12", name="r12")
            nc.vector.tensor_scalar(
                out=r12, in0=sq2[:, SE:W], scalar1=a4, scalar2=n4,
                op0=mybir.AluOpType.mult, op1=mybir.AluOpType.add)
            r22 = tpool.tile([D, SE], BF16, tag="r22", name="r22")
            nc.vector.tensor_scalar_mul(out=r22, in0=sp2[:, SE:W], scalar1=a4)

            # --- 32 accumulating matmuls: 4 term-pairs x (pair, i-half) ---
            terms = [
                (sin1, ecos1, cos1, esin1),
                (sp1, r11, cpp1, r21),
                (sin2, ecos2, cos2, esin2),
                (sp2, r12, cpp2, r22),
            ]
            ntt = len(terms)
            for t_i, (lhs_a, rhs_a, lhs_b, rhs_b) in enumerate(terms):
                for p in range(PAIRS):
                    for ih in range(2):
                        # s-region starts at col 0 in every lhs tile
                        sl = slice(p * L + ih * 128, p * L + (ih + 1) * 128)
                        ecs = slice(p * L, (p + 1) * L)
                        oc = slice(ih * L, (ih + 1) * L)
                        # start=True clears the WHOLE bank, so issue it only
                        # on the very first matmul into this acc tile; the
                        # other i-half's first write lands on has_written=0
                        # and overwrites (not accumulates) as needed.
                        nc.tensor.matmul(
                            acc2[p][:, oc], lhsT=lhs_a[:, sl],
                            rhs=rhs_a[:, ecs],
                            start=(t_i == 0 and ih == 0), stop=False)
                        nc.tensor.matmul(
                            acc2[p][:, oc], lhsT=lhs_b[:, sl],
                            rhs=rhs_b[:, ecs],
                            start=False, stop=(t_i == ntt - 1))

            # --- evacuate PSUM (1 DVE + 1 ACT copy) + output DMA ---
            for p in range(PAIRS):
                ev = setup.tile([128, 2, L], F32, tag=f"ev{p}", name="ev")
                if p == 0:
                    nc.vector.tensor_copy(out=ev, in_=acc2[p].rearrange(
                        "d (h j) -> d h j", h=2))
                else:
                    nc.scalar.copy(out=ev, in_=acc2[p].rearrange(
                        "d (h j) -> d h j", h=2))
                nc.sync.dma_start(
                    out=out_ext[p].rearrange("h il j -> il h j"), in_=ev)

        if repeat == 1:
            main_body()
        elif python_unroll:
            for _ in range(repeat):
                main_body()
        else:
            with tc.For_i(0, repeat, 1):
                main_body()
    nc.compile()
    return nc


_NC_CACHE = None


def kernel(start_hidden, end_hidden, v):
    global _NC_CACHE
    if _NC_CACHE is None:
        _NC_CACHE = build_nc()
    nc = _NC_CACHE

    sh = np.ascontiguousarray(start_hidden, dtype=np.float32).reshape(B * C, L, D)
    eh = np.ascontiguousarray(end_hidden, dtype=np.float32).reshape(B * C, L, D)
    v2 = np.ascontiguousarray(v, dtype=np.float32).reshape(D, 1)

    in_maps = [
        {
            "start_hidden": sh[k * PAIRS:(k + 1) * PAIRS],
            "end_hidden": eh[k * PAIRS:(k + 1) * PAIRS],
            "v": v2,
        }
        for k in range(N_CORES)
    ]

    res = None
    for attempt in range(3):
        try:
            res = run_bass_kernel_spmd(nc, in_maps, core_ids=list(range(N_CORES)))
            break
        except Exception:
            # transient NRT device-unrecoverable states clear on retry
            if attempt == 2:
                raise
            import time as _t
            _t.sleep(5)
    # per-core out: [PAIRS, 2, 128, L] = [p, ih, il, j] -> [p, i, j]
    per_core = [
        res.results[k]["out"].reshape(PAIRS, L, L)
        for k in range(N_CORES)
    ]
    full = np.concatenate(per_core, axis=0)  # [B*C, L(i), L(j)] in (b,c) order
    return np.ascontiguousarray(
        full.reshape(B, C, L, L).transpose(0, 2, 3, 1)
    ).astype(np.float32)
